# revision 61
# baseline (speedup 1.0000x reference)
"""Trainium2 Bass kernel for nn_EvMLP (segment_reduce EvNorm + invariant MLP).

Self-contained: hardcodes shapes/sharding. Accepts FULL inputs, returns FULL
output; shards the node dim N across 8 NeuronCores (pure data parallel).

Math (per row of ten [N, 592]):
  x10 = ten[:128]; eq = ten[128:]
  sumsq[c] = sum of eq^2 over segment c   (64x3, 32x5, 16x7 runs)
  d = sqrt(sumsq+1);  x11 = d-1;  x2 = eq / d[seg]
  x1 = [x10, x11]  (240)
  h = LN(x1@w1, g1, b1); h = silu(h@w2+b2); h = LN(h, g2, b2n); h = h@w3+b3
  out = [h, x2]

Implementation (v2, fp16 compute):
  - rows-on-partitions for the eq path; eq^2 written fp16 into a padded
    layout with a 1.0 slot per segment so the DVE segment reduce directly
    yields s1 = sumsq+1 (and runs in 16-bit mode)
  - rsqrt via bit-trick seed (i16 magic 0x59BA for fp16 / i32 0x5F3759DF for
    fp32 LN stats, computed on ACT) + ONE fused Newton step on DVE
  - MLP feature-major; all matmuls fp16 (4x faster PE than fp32); x10
    transposed via PE (fp32r) then copy-cast to fp16; x11 transposed via the
    DMA XBAR (16-bit transpose) straight into SBUF
  - LN mean folded into weights host-side; variance via ones/128 matmul;
    LN scales applied on DVE fused with the PSUM->SBUF move + fp16 cast
  - final bias b3 folded into the PE accumulation (rank-1 ones matmul)
"""
import sys

sys.path.insert(0, "/opt/trn_rl_repo")

import numpy as np

import concourse.bass as bass
import concourse.bacc as bacc
import concourse.tile as tile
from concourse import mybir
from concourse.bass_utils import run_bass_kernel_spmd

f32 = mybir.dt.float32
f32r = mybir.dt.float32r
f16 = mybir.dt.float16
i32 = mybir.dt.int32
i16 = mybir.dt.int16

# ---------------------------------------------------------------- constants --
N = 100000
DIM = 592
N_INV = 128
N_EQ_CH = 112
N_EQ = 464
EPS = 1e-5
N_CORES = 8
BLOCKS_PER_CORE = 98                      # 98*128 = 12544 rows/core
ROWS_PER_CORE = BLOCKS_PER_CORE * 128
NPAD = N_CORES * ROWS_PER_CORE            # 100352
MACROS = [8] * 12 + [2]                   # blocks per macro-tile (sum 98)
MAGIC = 0x5F3759DF
MAGICF = float(MAGIC)
MAGIC16F = 22970.0                        # fp16 rsqrt seed magic (0x59BA)

# segment groups: (n_channels, width, eq col offset, channel offset, padded
# offset) — padded layout stores w+1 slots per channel, last slot == 1.0
SEGS = [(64, 3, 0, 0, 0), (32, 5, 192, 64, 256), (16, 7, 352, 96, 448)]
EQ2_PAD = 576                             # sum of nch*(w+1)

_EXPECTED_REP = np.concatenate(
    [np.repeat(np.arange(m) + off, 2 * l + 1)
     for l, (m, off) in enumerate([(128, 0), (64, 128), (32, 192), (16, 224)])]
)

# ------------------------------------------------------------- custom DVE op --
from concourse.dve_spec import (
    Spec, Src0, Src1, C0, C1, C2, lower, scan as dve_scan, sq as dve_sq, AluOp,
)
from concourse.dve_uop import DveOpSpec
import concourse.dve_ops as dve_ops
from concourse.dve_ops import DveOp

# Newton rsqrt step: out = y*(C1 - C0*((v+C2)*y*y));  in0=v, in1=y
_nr_body = Src1 * (C1 - ((Src0 + C2) * (Src1 * Src1)) * C0)


def _nr_ref(in0, in1, s0, s1, imm2):
    y = in1.astype(np.float32)
    v = in0.astype(np.float32)
    return (y * (np.float32(s1) - ((v + np.float32(imm2)) * y * y) * np.float32(s0))
            ).astype(np.float32)


def _register_op(name, spec):
    if name in dve_ops._SUB_OPCODE_FOR_NAME:
        for op in dve_ops.OPS:
            if op.name == name:
                return op
    from concourse.dve_spec import _has_src1 as has_src1
    shas = {}
    row = 1 + len(dve_ops.OPS)
    for ver in ("v3", "v4"):
        s = DveOpSpec(name=name, opcode=row, uops=lower(spec, ver=ver),
                      rd1_en=has_src1(spec))
        shas[ver] = s.sha(ver)
    op = DveOp(name, spec, subdim=False, uops_sha=shas)
    dve_ops.OPS.append(op)
    dve_ops._SUB_OPCODE_FOR_NAME[name] = row
    dve_ops.CUSTOM_DVE_SPECS[name] = spec
    return op


RSQRT_NR = _register_op("ANT_RSQRT_NR2", Spec(body=_nr_body, reference=_nr_ref))


def _mk_mulsub1():
    from concourse.dve_spec import One
    return _register_op(
        "ANT_MUL_SUB1",
        Spec(
            body=(Src0 * Src1) - One,
            reference=lambda in0, in1, s0, s1, imm2: (
                in0.astype(np.float32) * in1 - np.float32(1.0)
            ).astype(np.float32),
        ),
    )


MUL_SUB1 = _mk_mulsub1()

# running prefix sum of squares along the free stream (segment sums are
# recovered by differencing at segment boundaries)
SQ_PSUM = _register_op(
    "ANT_SQ_PSUM",
    Spec(
        body=dve_scan(AluOp.ADD, dve_sq(Src0)),
        reference=lambda in0, in1, s0, s1, imm2: np.cumsum(
            (in0.astype(np.float32) ** 2).reshape(in0.shape[0], -1),
            axis=-1, dtype=np.float32,
        ).reshape(in0.shape).astype(np.float32),
    ),
)


# ------------------------------------------------------------ kernel builder --
def _build_nc():
    nc = bacc.Bacc()

    x = nc.dram_tensor("x", [ROWS_PER_CORE, DIM], f32, kind="ExternalInput")
    out = nc.dram_tensor("out", [ROWS_PER_CORE, DIM], f32, kind="ExternalOutput")
    w1a_d = nc.dram_tensor("w1a", [128, 128], f16, kind="ExternalInput")
    w1b_d = nc.dram_tensor("w1b", [128, 128], f16, kind="ExternalInput")
    w2_d = nc.dram_tensor("w2p", [128, 128], f16, kind="ExternalInput")
    w3_d = nc.dram_tensor("w3p", [128, 128], f16, kind="ExternalInput")
    cmat_d = nc.dram_tensor("cmat", [128, 128], f16, kind="ExternalInput")
    onesd_d = nc.dram_tensor("onesd", [128, 128], f16, kind="ExternalInput")
    ident_d = nc.dram_tensor("ident", [128, 128], f32, kind="ExternalInput")
    identh_d = nc.dram_tensor("identh", [128, 128], f16, kind="ExternalInput")
    ones1_d = nc.dram_tensor("ones1", [1, 128], f16, kind="ExternalInput")
    b3row_d = nc.dram_tensor("b3row", [1, 1024], f16, kind="ExternalInput")
    b2_d = nc.dram_tensor("b2c", [128, 1], f32, kind="ExternalInput")

    # float consts used as activation bias
    for _v in (MAGICF, MAGIC16F):
        _t = nc.alloc_sbuf_tensor(f"const-f32-{_v}", [128, 1], f32)
        nc.gpsimd.memset(_t.ap(), _v)
        nc.const_aps.aps[(f32, _v)] = _t.ap()
    nc.all_engine_barrier()

    AF = mybir.ActivationFunctionType
    ALU = mybir.AluOpType
    AX = mybir.AxisListType

    from contextlib import ExitStack

    with tile.TileContext(nc) as tc:
        with ExitStack() as ctx:
            wpool = ctx.enter_context(tc.tile_pool(name="w", bufs=1))
            xpool = ctx.enter_context(tc.tile_pool(name="xp", bufs=3))
            opool = ctx.enter_context(tc.tile_pool(name="op", bufs=4))
            spool = ctx.enter_context(tc.tile_pool(name="sp", bufs=2))
            cpool = ctx.enter_context(tc.tile_pool(name="cp", bufs=2))
            ps_tp = ctx.enter_context(tc.tile_pool(name="ptp", bufs=1, space="PSUM"))
            ps_mm = ctx.enter_context(tc.tile_pool(name="pmm", bufs=2, space="PSUM"))
            ps_q = ctx.enter_context(tc.tile_pool(name="pq", bufs=1, space="PSUM"))

            def wload(name, shape, dtype, dram):
                t = wpool.tile(shape, dtype, tag=name)
                nc.gpsimd.dma_start(out=t, in_=dram[:, :])
                return t

            w1a = wload("w1a", [128, 128], f16, w1a_d)
            w1b = wload("w1b", [128, 128], f16, w1b_d)
            w2p = wload("w2p", [128, 128], f16, w2_d)
            w3p = wload("w3p", [128, 128], f16, w3_d)
            cmat = wload("cmat", [128, 128], f16, cmat_d)
            onesd = wload("onesd", [128, 128], f16, onesd_d)
            ident = wload("ident", [128, 128], f32, ident_d)
            identh = wload("identh", [128, 128], f16, identh_d)
            ones1 = wload("ones1", [1, 128], f16, ones1_d)
            b3row = wload("b3row", [1, 1024], f16, b3row_d)
            b2c = wload("b2c", [128, 1], f32, b2_d)

            # persistent ping-pong tiles: eq^2 padded (1.0 slot per segment
            # folds the +1 into the reduce) and x11 padded to 128 channels
            # (16 zero channels hit zero rows of w1b)
            eq2s = [wpool.tile([128, 8, EQ2_PAD], f16, tag=f"eq2{i}",
                               name=f"eq2{i}") for i in range(1)]
            x11s = [wpool.tile([128, 8, 128], f16, tag=f"x11{i}",
                               name=f"x11{i}") for i in range(2)]
            for t in eq2s:
                for (nch, w, eqoff, choff, poff) in SEGS:
                    pw = w + 1
                    ones_ap = t[:, :, poff : poff + nch * pw].rearrange(
                        "p b (c t) -> p b c t", t=pw
                    )[:, :, :, w : w + 1]
                    nc.gpsimd.memset(ones_ap, 1.0)
            for t in x11s:
                nc.gpsimd.memset(t[:, :, N_EQ_CH:128], 0.0)

            # software pipeline: emit eq-path(k) interleaved with MLP(k-1) so
            # each engine's in-order queue holds independent work from two
            # macros and cross-engine stalls overlap
            row_starts = []
            acc = 0
            for nb in MACROS:
                row_starts.append(acc)
                acc += nb * 128

            live = {}

            def phaseA(mi, nb):
                row0 = row_starts[mi]
                R_rows = nb * 128
                xv = x[row0 : row0 + R_rows, :].rearrange("(p b) d -> p b d", b=nb)
                X = xpool.tile([128, nb, DIM], f32, tag="X", name="X")
                nc.sync.dma_start(out=X, in_=xv)
                live[("X", mi)] = X

            def phaseB(mi, nb):
                row0 = row_starts[mi]
                R_rows = nb * 128
                RR = R_rows

                x11 = x11s[mi % 2]
                eq2 = eq2s[0]

                X = live.pop(("X", mi))
                O = opool.tile([128, nb, DIM], f32, tag="O", name="O")

                def sq_in(g):
                    nch, w, eqoff, choff, poff = SEGS[g]
                    return X[:, :, N_INV + eqoff : N_INV + eqoff + nch * w].rearrange(
                        "p b (c t) -> p b c t", t=w
                    )

                def sq_out(g):
                    nch, w, eqoff, choff, poff = SEGS[g]
                    return eq2[:, 0:nb, poff : poff + nch * (w + 1)].rearrange(
                        "p b (c t) -> p b c t", t=w + 1
                    )[:, :, :, 0:w]

                # ---- eq path (rows on partitions) ----
                nc.scalar.activation(out=sq_out(0), in_=sq_in(0), func=AF.Square)
                for g in (1, 2):
                    nc.gpsimd.tensor_tensor(
                        out=sq_out(g), in0=sq_in(g), in1=sq_in(g), op=ALU.mult
                    )

                # segment reduce (fp16 in/out, 1.0 pad slot folds in the +1)
                s1 = spool.tile([128, nb, N_EQ_CH], f16, tag="s1")
                with nc.allow_low_precision("fp16 segment sumsq; tol 2e-2"):
                    for (nch, w, eqoff, choff, poff) in SEGS:
                        pw = w + 1
                        nc.vector.reduce_sum(
                            out=s1[:, :, choff : choff + nch],
                            in_=eq2[:, 0:nb, poff : poff + nch * pw].rearrange(
                                "p b (c t) -> p b c t", t=pw
                            ),
                            axis=AX.X,
                        )

                # r = rsqrt(s1): i16 bit-trick seed on ACT + 1 Newton on DVE
                seedb = spool.tile([128, nb, N_EQ_CH], i16, tag="seedb")
                nc.scalar.activation(
                    out=seedb, in_=s1.bitcast(i16), func=AF.Identity,
                    scale=-0.5, bias=MAGIC16F,
                )
                flat3 = lambda ap: ap.rearrange("p a b -> p (a b)")
                r = spool.tile([128, nb, N_EQ_CH], f16, tag="r")
                nc.vector._custom_dve(
                    RSQRT_NR, out=flat3(r), in0=flat3(s1),
                    in1=flat3(seedb.bitcast(f16)), s0=0.5, s1=1.5, imm2=0.0,
                )
                # x11 = s1*r - 1  (= sqrt(s1) - 1), fp16, into padded tile
                nc.vector._custom_dve(
                    MUL_SUB1, out=x11[:, 0:nb, 0:N_EQ_CH], in0=s1,
                    in1=r, s0=0.0, s1=0.0, imm2=0.0,
                )

                # x2 = eq * r[seg] -> O[:, :, 128:]  (fp32 out for DMA);
                # all on GP - DVE is saturated by reduce/Newton/PSUM scales
                for g, eng in ((0, nc.gpsimd), (1, nc.gpsimd), (2, nc.gpsimd)):
                    nch, w, eqoff, choff, poff = SEGS[g]
                    rbc = (
                        r[:, :, choff : choff + nch]
                        .unsqueeze(-1)
                        .broadcast_to((128, nb, nch, w))
                    )
                    eng.tensor_tensor(
                        out=O[:, :, N_INV + eqoff : N_INV + eqoff + nch * w].rearrange(
                            "p b (c t) -> p b c t", t=w
                        ),
                        in0=sq_in(g),
                        in1=rbc,
                        op=ALU.mult,
                    )
                live[("O", mi)] = O

                # x10^T via PE into a macro PSUM tile, one ACT copy-cast; then
                # x11^T (fp16) into the same PSUM ring slot, one ACT copy
                TPa = ps_tp.tile([128, RR], f32, tag="tp", name="TPa")
                for b in range(nb):
                    nc.tensor.transpose(
                        TPa[:, b * 128 : (b + 1) * 128], X[:, b, 0:N_INV], ident
                    )
                x1Ta = cpool.tile([128, RR], f16, tag="x1Ta", name="x1Ta")
                nc.scalar.activation(out=x1Ta, in_=TPa, func=AF.Identity)

                TPb = ps_tp.tile([128, RR], f16, tag="tp", name="TPb")
                for b in range(nb):
                    nc.tensor.transpose(
                        TPb[:, b * 128 : (b + 1) * 128], x11[:, b, :], identh
                    )
                x1Tb = cpool.tile([128, RR], f16, tag="x1Tb", name="x1Tb")
                nc.scalar.activation(out=x1Tb, in_=TPb, func=AF.Identity)

                live[mi] = (nb, x1Ta, x1Tb)

            def phaseC(mi, nb):
                RR = nb * 128
                _, x1Ta, x1Tb = live[mi]
                nbanks = (RR + 511) // 512

                # H1 = w1a^T x10^T + w1b^T x11^T   [128, RR] PSUM f32
                # (grouped by stationary weight so PE can reuse loads)
                H1 = ps_mm.tile([128, RR], f32, tag="mm", name="H1")
                for c in range(nbanks):
                    lo = c * 512
                    hi = min(RR, lo + 512)
                    nc.tensor.matmul(
                        H1[:, lo:hi], w1a, x1Ta[:, lo:hi], start=True, stop=False
                    )
                for c in range(nbanks):
                    lo = c * 512
                    hi = min(RR, lo + 512)
                    nc.tensor.matmul(
                        H1[:, lo:hi], w1b, x1Tb[:, lo:hi],
                        start=False, stop=True,
                    )

                # LN1 stats: sq1 -> Q1 = mean(sq1); rstd1 = rsqrt(Q1+eps)
                sq1 = cpool.tile([128, RR], f16, tag="sq1")
                nc.scalar.activation(out=sq1, in_=H1, func=AF.Square)
                Q1 = ps_q.tile([128, RR], f32, tag="q")
                for c in range(nbanks):
                    lo = c * 512
                    hi = min(RR, lo + 512)
                    nc.tensor.matmul(
                        Q1[:, lo:hi], onesd, sq1[:, lo:hi], start=True, stop=True
                    )
                sd1 = cpool.tile([128, RR], i32, tag="sd1", bufs=1)
                nc.scalar.activation(out=sd1, in_=Q1.bitcast(i32),
                                     func=AF.Identity, scale=-0.5, bias=MAGICF)
                rstd1 = cpool.tile([128, RR], f16, tag="rstd1")
                nc.vector._custom_dve(
                    RSQRT_NR, out=rstd1, in0=Q1, in1=sd1.bitcast(f32),
                    s0=0.5, s1=1.5, imm2=float(EPS),
                )
                # hm1 = H1 * rstd1 (fused PSUM->SBUF move + fp16 cast)
                hm1 = cpool.tile([128, RR], f16, tag="hm1")
                nc.vector.tensor_tensor(out=hm1, in0=H1, in1=rstd1, op=ALU.mult)
                live[mi] = (nb, hm1)

            def phaseD(mi, nb):
                RR = nb * 128
                _, hm1 = live.pop(mi)
                nbanks = (RR + 511) // 512

                # H2 = w2p^T hm1 ; avs = silu(H2 + b2c)
                H2 = ps_mm.tile([128, RR], f32, tag="mm")
                for c in range(nbanks):
                    lo = c * 512
                    hi = min(RR, lo + 512)
                    nc.tensor.matmul(
                        H2[:, lo:hi], w2p, hm1[:, lo:hi], start=True, stop=True
                    )
                avs = cpool.tile([128, RR], f16, tag="avs")
                nc.scalar.activation(out=avs, in_=H2, func=AF.Silu, bias=b2c)

                # AC = cmat^T avs (mean-centered); LN2 stats
                AC = ps_mm.tile([128, RR], f32, tag="mm")
                for c in range(nbanks):
                    lo = c * 512
                    hi = min(RR, lo + 512)
                    nc.tensor.matmul(
                        AC[:, lo:hi], cmat, avs[:, lo:hi], start=True, stop=True
                    )
                sq2 = cpool.tile([128, RR], f16, tag="sq2")
                nc.scalar.activation(out=sq2, in_=AC, func=AF.Square)
                Q2 = ps_q.tile([128, RR], f32, tag="q")
                for c in range(nbanks):
                    lo = c * 512
                    hi = min(RR, lo + 512)
                    nc.tensor.matmul(
                        Q2[:, lo:hi], onesd, sq2[:, lo:hi], start=True, stop=True
                    )
                sd2 = cpool.tile([128, RR], i32, tag="sd2", bufs=1)
                nc.scalar.activation(out=sd2, in_=Q2.bitcast(i32),
                                     func=AF.Identity, scale=-0.5, bias=MAGICF)
                rstd2 = cpool.tile([128, RR], f16, tag="rstd2")
                nc.vector._custom_dve(
                    RSQRT_NR, out=rstd2, in0=Q2, in1=sd2.bitcast(f32),
                    s0=0.5, s1=1.5, imm2=float(EPS),
                )
                hn2 = cpool.tile([128, RR], f16, tag="hn2")
                nc.vector.tensor_tensor(out=hn2, in0=AC, in1=rstd2, op=ALU.mult)
                live[("hn2", mi)] = hn2

            def phaseE(mi, nb):
                row0 = row_starts[mi]
                R_rows = nb * 128
                RR = R_rows
                hn2 = live.pop(("hn2", mi))
                nbanks = (RR + 511) // 512

                # H3 natural orientation: bias via rank-1 ones matmul, then
                # per-block lhsT=hn2 matmuls accumulate on top
                H3n = ps_q.tile([128, RR], f32, tag="q")
                for c in range(nbanks):
                    lo = c * 512
                    hi = min(RR, lo + 512)
                    nc.tensor.matmul(
                        H3n[:, lo:hi], ones1, b3row[:, lo:hi],
                        start=True, stop=False,
                    )
                    for j in range(lo // 128, hi // 128):
                        nc.tensor.matmul(
                            H3n[:, j * 128 : (j + 1) * 128],
                            hn2[:, j * 128 : (j + 1) * 128], w3p,
                            start=False, stop=True,
                        )
                O = live.pop(("O", mi))
                nc.scalar.activation(
                    out=O[:, :, 0:N_INV],
                    in_=H3n.rearrange("p (b j) -> p b j", j=128),
                    func=AF.Identity,
                )
                ov = out[row0 : row0 + R_rows, :].rearrange("(p b) d -> p b d", b=nb)
                nc.gpsimd.dma_start(out=ov, in_=O)

            # 5-deep software pipeline, oldest work emitted first; every
            # cross-phase dependency is >= 1 iteration old so each engine's
            # in-order queue streams without same-iteration stalls
            nmac = len(MACROS)
            for t in range(nmac + 4):
                if 4 <= t:
                    phaseE(t - 4, MACROS[t - 4])
                if 3 <= t <= nmac + 2:
                    phaseD(t - 3, MACROS[t - 3])
                if 2 <= t <= nmac + 1:
                    phaseC(t - 2, MACROS[t - 2])
                if 1 <= t <= nmac:
                    phaseB(t - 1, MACROS[t - 1])
                if t < nmac:
                    phaseA(t, MACROS[t])

    nc.finalize()
    return nc


_NC_CACHE = {}


def _get_nc():
    if "nc" not in _NC_CACHE:
        _NC_CACHE["nc"] = _build_nc()
    return _NC_CACHE["nc"]


# --------------------------------------------------------------- host driver --
def _prep_weights(w1, g1, beta1, w2, b2, g2, beta2, w3, b3):
    C = np.eye(128, dtype=np.float64) - 1.0 / 128.0
    w1p = w1.astype(np.float64) @ C                       # [240,128]
    w2p = (g1.astype(np.float64)[:, None] * w2.astype(np.float64))
    b2c = beta1.astype(np.float64) @ w2.astype(np.float64) + b2.astype(np.float64)
    w3p = (g2.astype(np.float64)[:, None] * w3.astype(np.float64))
    b3c = beta2.astype(np.float64) @ w3.astype(np.float64) + b3.astype(np.float64)
    w1b_pad = np.zeros((128, 128), dtype=np.float64)
    w1b_pad[0:N_EQ_CH] = w1p[128:240]
    return {
        "w1a": np.ascontiguousarray(w1p[0:128]).astype(np.float16),
        "w1b": w1b_pad.astype(np.float16),
        "w2p": w2p.astype(np.float16),
        "w3p": w3p.astype(np.float16),
        "cmat": C.astype(np.float16),
        "onesd": np.full((128, 128), 1.0 / 128.0, dtype=np.float16),
        "ident": np.eye(128, dtype=np.float32),
        "identh": np.eye(128, dtype=np.float16),
        "ones1": np.ones((1, 128), dtype=np.float16),
        "b3row": np.tile(b3c, 8)[None, :].astype(np.float16),
        "b2c": b2c.astype(np.float32).reshape(128, 1),
    }


def _np_reference(ten, w1, g1, beta1, w2, b2, g2, beta2, w3, b3):
    """Pure-numpy fallback (used only if rep_layout is unexpected)."""
    x10 = ten[:, :N_INV]
    eq = ten[:, N_INV:]
    sumsq = np.zeros((ten.shape[0], N_EQ_CH), np.float32)
    for (nch, w, eqoff, choff, poff) in SEGS:
        sumsq[:, choff:choff + nch] = (
            (eq[:, eqoff:eqoff + nch * w].reshape(-1, nch, w) ** 2).sum(-1)
        )
    d = np.sqrt(sumsq + 1.0)
    x11 = d - 1.0
    x1 = np.concatenate([x10, x11], 1)
    seg = np.concatenate([np.repeat(np.arange(nch) + choff, w)
                          for (nch, w, eqoff, choff, poff) in SEGS])
    x2 = eq / d[:, seg]

    def ln(h, g, b):
        mu = h.mean(-1, keepdims=True)
        var = h.var(-1, keepdims=True)
        return (h - mu) / np.sqrt(var + EPS) * g + b

    h = x1 @ w1
    h = ln(h, g1, beta1)
    h = h @ w2 + b2
    h = h * (1.0 / (1.0 + np.exp(-h)))
    h = ln(h, g2, beta2)
    h = h @ w3 + b3
    return np.concatenate([h, x2], 1).astype(np.float32)


def kernel(ten, rep_layout, w1, g1, beta1, w2, b2, g2, beta2, w3, b3):
    ten = np.asarray(ten, dtype=np.float32)
    args = [np.asarray(a) for a in (w1, g1, beta1, w2, b2, g2, beta2, w3, b3)]
    w1, g1, beta1, w2, b2, g2, beta2, w3, b3 = [a.astype(np.float32) for a in args]

    if not np.array_equal(np.asarray(rep_layout).astype(np.int64), _EXPECTED_REP):
        return _np_reference(ten, w1, g1, beta1, w2, b2, g2, beta2, w3, b3)

    wmap = _prep_weights(w1, g1, beta1, w2, b2, g2, beta2, w3, b3)

    xpad = np.zeros((NPAD, DIM), dtype=np.float32)
    xpad[:N] = ten
    shards = xpad.reshape(N_CORES, ROWS_PER_CORE, DIM)

    nc = _get_nc()
    in_maps = [dict(wmap, x=np.ascontiguousarray(shards[c]))
               for c in range(N_CORES)]
    last_err = None
    for _attempt in range(3):
        try:
            res = run_bass_kernel_spmd(nc, in_maps, list(range(N_CORES))).results
            break
        except Exception as e:  # transient device-unrecoverable errors
            last_err = e
            import time as _time
            _time.sleep(10)
    else:
        raise last_err
    outp = np.concatenate([res[c]["out"] for c in range(N_CORES)], axis=0)
    return np.ascontiguousarray(outp[:N])


# revision 63
# speedup vs baseline: 1.0374x; 1.0374x over previous
"""Trainium2 Bass kernel for nn_EvMLP (segment_reduce EvNorm + invariant MLP).

Self-contained: hardcodes shapes/sharding. Accepts FULL inputs, returns FULL
output; shards the node dim N across 8 NeuronCores (pure data parallel).

Math (per row of ten [N, 592]):
  x10 = ten[:128]; eq = ten[128:]
  sumsq[c] = sum of eq^2 over segment c   (64x3, 32x5, 16x7 runs)
  d = sqrt(sumsq+1);  x11 = d-1;  x2 = eq / d[seg]
  x1 = [x10, x11]  (240)
  h = LN(x1@w1, g1, b1); h = silu(h@w2+b2); h = LN(h, g2, b2n); h = h@w3+b3
  out = [h, x2]

Implementation (v2, fp16 compute):
  - rows-on-partitions for the eq path; eq^2 written fp16 into a padded
    layout with a 1.0 slot per segment so the DVE segment reduce directly
    yields s1 = sumsq+1 (and runs in 16-bit mode)
  - rsqrt via bit-trick seed (i16 magic 0x59BA for fp16 / i32 0x5F3759DF for
    fp32 LN stats, computed on ACT) + ONE fused Newton step on DVE
  - MLP feature-major; all matmuls fp16 (4x faster PE than fp32); x10
    transposed via PE (fp32r) then copy-cast to fp16; x11 transposed via the
    DMA XBAR (16-bit transpose) straight into SBUF
  - LN mean folded into weights host-side; variance via ones/128 matmul;
    LN scales applied on DVE fused with the PSUM->SBUF move + fp16 cast
  - final bias b3 folded into the PE accumulation (rank-1 ones matmul)
"""
import sys

sys.path.insert(0, "/opt/trn_rl_repo")

import numpy as np

import concourse.bass as bass
import concourse.bacc as bacc
import concourse.tile as tile
from concourse import mybir
from concourse.bass_utils import run_bass_kernel_spmd

f32 = mybir.dt.float32
f32r = mybir.dt.float32r
f16 = mybir.dt.float16
i32 = mybir.dt.int32
i16 = mybir.dt.int16

# ---------------------------------------------------------------- constants --
N = 100000
DIM = 592
N_INV = 128
N_EQ_CH = 112
N_EQ = 464
EPS = 1e-5
N_CORES = 8
BLOCKS_PER_CORE = 98                      # 98*128 = 12544 rows/core
ROWS_PER_CORE = BLOCKS_PER_CORE * 128
NPAD = N_CORES * ROWS_PER_CORE            # 100352
MACROS = [8] * 12 + [2]                   # blocks per macro-tile (sum 98)
MAGIC = 0x5F3759DF
MAGICF = float(MAGIC)
MAGIC16F = 22970.0                        # fp16 rsqrt seed magic (0x59BA)

# segment groups: (n_channels, width, eq col offset, channel offset, padded
# offset) — padded layout stores w+1 slots per channel, last slot == 1.0
SEGS = [(64, 3, 0, 0, 0), (32, 5, 192, 64, 256), (16, 7, 352, 96, 448)]
EQ2_PAD = 576                             # sum of nch*(w+1)

_EXPECTED_REP = np.concatenate(
    [np.repeat(np.arange(m) + off, 2 * l + 1)
     for l, (m, off) in enumerate([(128, 0), (64, 128), (32, 192), (16, 224)])]
)

# ------------------------------------------------------------- custom DVE op --
from concourse.dve_spec import (
    Spec, Src0, Src1, C0, C1, C2, lower, scan as dve_scan, sq as dve_sq, AluOp,
)
from concourse.dve_uop import DveOpSpec
import concourse.dve_ops as dve_ops
from concourse.dve_ops import DveOp

# Newton rsqrt step: out = y*(C1 - C0*((v+C2)*y*y));  in0=v, in1=y
_nr_body = Src1 * (C1 - ((Src0 + C2) * (Src1 * Src1)) * C0)


def _nr_ref(in0, in1, s0, s1, imm2):
    y = in1.astype(np.float32)
    v = in0.astype(np.float32)
    return (y * (np.float32(s1) - ((v + np.float32(imm2)) * y * y) * np.float32(s0))
            ).astype(np.float32)


def _register_op(name, spec):
    if name in dve_ops._SUB_OPCODE_FOR_NAME:
        for op in dve_ops.OPS:
            if op.name == name:
                return op
    from concourse.dve_spec import _has_src1 as has_src1
    shas = {}
    row = 1 + len(dve_ops.OPS)
    for ver in ("v3", "v4"):
        s = DveOpSpec(name=name, opcode=row, uops=lower(spec, ver=ver),
                      rd1_en=has_src1(spec))
        shas[ver] = s.sha(ver)
    op = DveOp(name, spec, subdim=False, uops_sha=shas)
    dve_ops.OPS.append(op)
    dve_ops._SUB_OPCODE_FOR_NAME[name] = row
    dve_ops.CUSTOM_DVE_SPECS[name] = spec
    return op


RSQRT_NR = _register_op("ANT_RSQRT_NR2", Spec(body=_nr_body, reference=_nr_ref))


def _mk_mulsub1():
    from concourse.dve_spec import One
    return _register_op(
        "ANT_MUL_SUB1",
        Spec(
            body=(Src0 * Src1) - One,
            reference=lambda in0, in1, s0, s1, imm2: (
                in0.astype(np.float32) * in1 - np.float32(1.0)
            ).astype(np.float32),
        ),
    )


MUL_SUB1 = _mk_mulsub1()

# running prefix sum of squares along the free stream (segment sums are
# recovered by differencing at segment boundaries)
SQ_PSUM = _register_op(
    "ANT_SQ_PSUM",
    Spec(
        body=dve_scan(AluOp.ADD, dve_sq(Src0)),
        reference=lambda in0, in1, s0, s1, imm2: np.cumsum(
            (in0.astype(np.float32) ** 2).reshape(in0.shape[0], -1),
            axis=-1, dtype=np.float32,
        ).reshape(in0.shape).astype(np.float32),
    ),
)


# ------------------------------------------------------------ kernel builder --
def _build_nc():
    nc = bacc.Bacc()

    x = nc.dram_tensor("x", [ROWS_PER_CORE, DIM], f32, kind="ExternalInput")
    out = nc.dram_tensor("out", [ROWS_PER_CORE, DIM], f32, kind="ExternalOutput")
    w1a_d = nc.dram_tensor("w1a", [128, 128], f16, kind="ExternalInput")
    w1b_d = nc.dram_tensor("w1b", [128, 128], f16, kind="ExternalInput")
    w2_d = nc.dram_tensor("w2p", [128, 128], f16, kind="ExternalInput")
    w3_d = nc.dram_tensor("w3p", [128, 128], f16, kind="ExternalInput")
    cmat_d = nc.dram_tensor("cmat", [128, 128], f16, kind="ExternalInput")
    onesd_d = nc.dram_tensor("onesd", [128, 128], f16, kind="ExternalInput")
    ident_d = nc.dram_tensor("ident", [128, 128], f32, kind="ExternalInput")
    identh_d = nc.dram_tensor("identh", [128, 128], f16, kind="ExternalInput")
    ones1_d = nc.dram_tensor("ones1", [1, 128], f16, kind="ExternalInput")
    b3row_d = nc.dram_tensor("b3row", [1, 1024], f16, kind="ExternalInput")
    b2_d = nc.dram_tensor("b2c", [128, 1], f32, kind="ExternalInput")

    # float consts used as activation bias
    for _v in (MAGICF, MAGIC16F):
        _t = nc.alloc_sbuf_tensor(f"const-f32-{_v}", [128, 1], f32)
        nc.gpsimd.memset(_t.ap(), _v)
        nc.const_aps.aps[(f32, _v)] = _t.ap()
    nc.all_engine_barrier()

    AF = mybir.ActivationFunctionType
    ALU = mybir.AluOpType
    AX = mybir.AxisListType

    from contextlib import ExitStack

    with tile.TileContext(nc) as tc:
        with ExitStack() as ctx:
            wpool = ctx.enter_context(tc.tile_pool(name="w", bufs=1))
            xpool = ctx.enter_context(tc.tile_pool(name="xp", bufs=3))
            opool = ctx.enter_context(tc.tile_pool(name="op", bufs=4))
            spool = ctx.enter_context(tc.tile_pool(name="sp", bufs=2))
            cpool = ctx.enter_context(tc.tile_pool(name="cp", bufs=2))
            ps_tp = ctx.enter_context(tc.tile_pool(name="ptp", bufs=1, space="PSUM"))
            ps_mm = ctx.enter_context(tc.tile_pool(name="pmm", bufs=2, space="PSUM"))
            ps_q = ctx.enter_context(tc.tile_pool(name="pq", bufs=1, space="PSUM"))

            def wload(name, shape, dtype, dram):
                t = wpool.tile(shape, dtype, tag=name)
                nc.gpsimd.dma_start(out=t, in_=dram[:, :])
                return t

            w1a = wload("w1a", [128, 128], f16, w1a_d)
            w1b = wload("w1b", [128, 128], f16, w1b_d)
            w2p = wload("w2p", [128, 128], f16, w2_d)
            w3p = wload("w3p", [128, 128], f16, w3_d)
            cmat = wload("cmat", [128, 128], f16, cmat_d)
            onesd = wload("onesd", [128, 128], f16, onesd_d)
            ident = wload("ident", [128, 128], f32, ident_d)
            identh = wload("identh", [128, 128], f16, identh_d)
            ones1 = wload("ones1", [1, 128], f16, ones1_d)
            b3row = wload("b3row", [1, 1024], f16, b3row_d)
            b2c = wload("b2c", [128, 1], f32, b2_d)

            # persistent ping-pong tiles: eq^2 padded (1.0 slot per segment
            # folds the +1 into the reduce) and x11 padded to 128 channels
            # (16 zero channels hit zero rows of w1b)
            eq2s = [wpool.tile([128, 8, EQ2_PAD], f16, tag=f"eq2{i}",
                               name=f"eq2{i}") for i in range(1)]
            x11s = [wpool.tile([128, 8, 128], f16, tag=f"x11{i}",
                               name=f"x11{i}") for i in range(2)]
            for t in eq2s:
                for (nch, w, eqoff, choff, poff) in SEGS:
                    pw = w + 1
                    ones_ap = t[:, :, poff : poff + nch * pw].rearrange(
                        "p b (c t) -> p b c t", t=pw
                    )[:, :, :, w : w + 1]
                    nc.gpsimd.memset(ones_ap, 1.0)
            for t in x11s:
                nc.gpsimd.memset(t[:, :, N_EQ_CH:128], 0.0)

            # software pipeline: emit eq-path(k) interleaved with MLP(k-1) so
            # each engine's in-order queue holds independent work from two
            # macros and cross-engine stalls overlap
            row_starts = []
            acc = 0
            for nb in MACROS:
                row_starts.append(acc)
                acc += nb * 128

            live = {}

            def phaseA(mi, nb):
                row0 = row_starts[mi]
                R_rows = nb * 128
                xv = x[row0 : row0 + R_rows, :].rearrange("(p b) d -> p b d", b=nb)
                X = xpool.tile([128, nb, DIM], f32, tag="X", name="X")
                nc.sync.dma_start(out=X, in_=xv)
                live[("X", mi)] = X

            def phaseB(mi, nb):
                row0 = row_starts[mi]
                R_rows = nb * 128
                RR = R_rows

                x11 = x11s[mi % 2]
                eq2 = eq2s[0]

                X = live.pop(("X", mi))
                O = opool.tile([128, nb, DIM], f32, tag="O", name="O")

                def sq_in(g):
                    nch, w, eqoff, choff, poff = SEGS[g]
                    return X[:, :, N_INV + eqoff : N_INV + eqoff + nch * w].rearrange(
                        "p b (c t) -> p b c t", t=w
                    )

                def sq_out(g):
                    nch, w, eqoff, choff, poff = SEGS[g]
                    return eq2[:, 0:nb, poff : poff + nch * (w + 1)].rearrange(
                        "p b (c t) -> p b c t", t=w + 1
                    )[:, :, :, 0:w]

                # ---- eq path (rows on partitions) ----
                for g in (0, 1):
                    nc.scalar.activation(out=sq_out(g), in_=sq_in(g), func=AF.Square)
                nc.gpsimd.tensor_tensor(
                    out=sq_out(2), in0=sq_in(2), in1=sq_in(2), op=ALU.mult
                )

                # segment reduce (fp16 in/out, 1.0 pad slot folds in the +1)
                s1 = spool.tile([128, nb, N_EQ_CH], f16, tag="s1")
                with nc.allow_low_precision("fp16 segment sumsq; tol 2e-2"):
                    for (nch, w, eqoff, choff, poff) in SEGS:
                        pw = w + 1
                        nc.vector.reduce_sum(
                            out=s1[:, :, choff : choff + nch],
                            in_=eq2[:, 0:nb, poff : poff + nch * pw].rearrange(
                                "p b (c t) -> p b c t", t=pw
                            ),
                            axis=AX.X,
                        )

                # r = rsqrt(s1): i16 bit-trick seed on ACT + 1 Newton on DVE
                seedb = spool.tile([128, nb, N_EQ_CH], i16, tag="seedb")
                nc.scalar.activation(
                    out=seedb, in_=s1.bitcast(i16), func=AF.Identity,
                    scale=-0.5, bias=MAGIC16F,
                )
                flat3 = lambda ap: ap.rearrange("p a b -> p (a b)")
                r = spool.tile([128, nb, N_EQ_CH], f16, tag="r")
                nc.vector._custom_dve(
                    RSQRT_NR, out=flat3(r), in0=flat3(s1),
                    in1=flat3(seedb.bitcast(f16)), s0=0.5, s1=1.5, imm2=0.0,
                )
                # x11 = s1*r - 1  (= sqrt(s1) - 1), fp16, into padded tile
                nc.vector._custom_dve(
                    MUL_SUB1, out=x11[:, 0:nb, 0:N_EQ_CH], in0=s1,
                    in1=r, s0=0.0, s1=0.0, imm2=0.0,
                )

                # x2 = eq * r[seg] -> O[:, :, 128:]  (fp32 out for DMA)
                for g, eng in ((0, nc.vector), (1, nc.gpsimd), (2, nc.gpsimd)):
                    nch, w, eqoff, choff, poff = SEGS[g]
                    rbc = (
                        r[:, :, choff : choff + nch]
                        .unsqueeze(-1)
                        .broadcast_to((128, nb, nch, w))
                    )
                    eng.tensor_tensor(
                        out=O[:, :, N_INV + eqoff : N_INV + eqoff + nch * w].rearrange(
                            "p b (c t) -> p b c t", t=w
                        ),
                        in0=sq_in(g),
                        in1=rbc,
                        op=ALU.mult,
                    )
                live[("O", mi)] = O

                # x10^T via PE into a macro PSUM tile, one ACT copy-cast; then
                # x11^T (fp16) into the same PSUM ring slot, one ACT copy
                TPa = ps_tp.tile([128, RR], f32, tag="tp", name="TPa")
                for b in range(nb):
                    nc.tensor.transpose(
                        TPa[:, b * 128 : (b + 1) * 128], X[:, b, 0:N_INV], ident
                    )
                x1Ta = cpool.tile([128, RR], f16, tag="x1Ta", name="x1Ta")
                nc.scalar.activation(out=x1Ta, in_=TPa, func=AF.Identity)

                TPb = ps_tp.tile([128, RR], f16, tag="tp", name="TPb")
                for b in range(nb):
                    nc.tensor.transpose(
                        TPb[:, b * 128 : (b + 1) * 128], x11[:, b, :], identh
                    )
                x1Tb = cpool.tile([128, RR], f16, tag="x1Tb", name="x1Tb")
                nc.scalar.activation(out=x1Tb, in_=TPb, func=AF.Identity)

                live[mi] = (nb, x1Ta, x1Tb)

            def phaseC(mi, nb):
                RR = nb * 128
                _, x1Ta, x1Tb = live[mi]
                nbanks = (RR + 511) // 512

                # H1 = w1a^T x10^T + w1b^T x11^T   [128, RR] PSUM f32
                # (grouped by stationary weight so PE can reuse loads)
                H1 = ps_mm.tile([128, RR], f32, tag="mm", name="H1")
                for c in range(nbanks):
                    lo = c * 512
                    hi = min(RR, lo + 512)
                    nc.tensor.matmul(
                        H1[:, lo:hi], w1a, x1Ta[:, lo:hi], start=True, stop=False
                    )
                for c in range(nbanks):
                    lo = c * 512
                    hi = min(RR, lo + 512)
                    nc.tensor.matmul(
                        H1[:, lo:hi], w1b, x1Tb[:, lo:hi],
                        start=False, stop=True,
                    )

                # LN1 stats: sq1 -> Q1 = mean(sq1); rstd1 = rsqrt(Q1+eps)
                sq1 = cpool.tile([128, RR], f16, tag="sq1")
                nc.scalar.activation(out=sq1, in_=H1, func=AF.Square)
                Q1 = ps_q.tile([128, RR], f32, tag="q")
                for c in range(nbanks):
                    lo = c * 512
                    hi = min(RR, lo + 512)
                    nc.tensor.matmul(
                        Q1[:, lo:hi], onesd, sq1[:, lo:hi], start=True, stop=True
                    )
                sd1 = cpool.tile([128, RR], i32, tag="sd1", bufs=1)
                nc.scalar.activation(out=sd1, in_=Q1.bitcast(i32),
                                     func=AF.Identity, scale=-0.5, bias=MAGICF)
                rstd1 = cpool.tile([128, RR], f16, tag="rstd1")
                nc.vector._custom_dve(
                    RSQRT_NR, out=rstd1, in0=Q1, in1=sd1.bitcast(f32),
                    s0=0.5, s1=1.5, imm2=float(EPS),
                )
                # hm1 = H1 * rstd1 (fused PSUM->SBUF move + fp16 cast)
                hm1 = cpool.tile([128, RR], f16, tag="hm1")
                nc.vector.tensor_tensor(out=hm1, in0=H1, in1=rstd1, op=ALU.mult)
                live[mi] = (nb, hm1)

            def phaseD(mi, nb):
                RR = nb * 128
                _, hm1 = live.pop(mi)
                nbanks = (RR + 511) // 512

                # H2 = w2p^T hm1 ; avs = silu(H2 + b2c)
                H2 = ps_mm.tile([128, RR], f32, tag="mm")
                for c in range(nbanks):
                    lo = c * 512
                    hi = min(RR, lo + 512)
                    nc.tensor.matmul(
                        H2[:, lo:hi], w2p, hm1[:, lo:hi], start=True, stop=True
                    )
                avs = cpool.tile([128, RR], f16, tag="avs")
                nc.scalar.activation(out=avs, in_=H2, func=AF.Silu, bias=b2c)

                # AC = cmat^T avs (mean-centered); LN2 stats
                AC = ps_mm.tile([128, RR], f32, tag="mm")
                for c in range(nbanks):
                    lo = c * 512
                    hi = min(RR, lo + 512)
                    nc.tensor.matmul(
                        AC[:, lo:hi], cmat, avs[:, lo:hi], start=True, stop=True
                    )
                sq2 = cpool.tile([128, RR], f16, tag="sq2")
                nc.scalar.activation(out=sq2, in_=AC, func=AF.Square)
                Q2 = ps_q.tile([128, RR], f32, tag="q")
                for c in range(nbanks):
                    lo = c * 512
                    hi = min(RR, lo + 512)
                    nc.tensor.matmul(
                        Q2[:, lo:hi], onesd, sq2[:, lo:hi], start=True, stop=True
                    )
                sd2 = cpool.tile([128, RR], i32, tag="sd2", bufs=1)
                nc.scalar.activation(out=sd2, in_=Q2.bitcast(i32),
                                     func=AF.Identity, scale=-0.5, bias=MAGICF)
                rstd2 = cpool.tile([128, RR], f16, tag="rstd2")
                nc.vector._custom_dve(
                    RSQRT_NR, out=rstd2, in0=Q2, in1=sd2.bitcast(f32),
                    s0=0.5, s1=1.5, imm2=float(EPS),
                )
                hn2 = cpool.tile([128, RR], f16, tag="hn2")
                nc.vector.tensor_tensor(out=hn2, in0=AC, in1=rstd2, op=ALU.mult)
                live[("hn2", mi)] = hn2

            def phaseE(mi, nb):
                row0 = row_starts[mi]
                R_rows = nb * 128
                RR = R_rows
                hn2 = live.pop(("hn2", mi))
                nbanks = (RR + 511) // 512

                # H3 natural orientation: bias via rank-1 ones matmul, then
                # per-block lhsT=hn2 matmuls accumulate on top
                H3n = ps_q.tile([128, RR], f32, tag="q")
                for c in range(nbanks):
                    lo = c * 512
                    hi = min(RR, lo + 512)
                    nc.tensor.matmul(
                        H3n[:, lo:hi], ones1, b3row[:, lo:hi],
                        start=True, stop=False,
                    )
                    for j in range(lo // 128, hi // 128):
                        nc.tensor.matmul(
                            H3n[:, j * 128 : (j + 1) * 128],
                            hn2[:, j * 128 : (j + 1) * 128], w3p,
                            start=False, stop=True,
                        )
                O = live.pop(("O", mi))
                nc.scalar.activation(
                    out=O[:, :, 0:N_INV],
                    in_=H3n.rearrange("p (b j) -> p b j", j=128),
                    func=AF.Identity,
                )
                ov = out[row0 : row0 + R_rows, :].rearrange("(p b) d -> p b d", b=nb)
                nc.gpsimd.dma_start(out=ov, in_=O)

            # 5-deep software pipeline, oldest work emitted first; every
            # cross-phase dependency is >= 1 iteration old so each engine's
            # in-order queue streams without same-iteration stalls
            nmac = len(MACROS)
            for t in range(nmac + 4):
                if 4 <= t:
                    phaseE(t - 4, MACROS[t - 4])
                if 3 <= t <= nmac + 2:
                    phaseD(t - 3, MACROS[t - 3])
                if 2 <= t <= nmac + 1:
                    phaseC(t - 2, MACROS[t - 2])
                if 1 <= t <= nmac:
                    phaseB(t - 1, MACROS[t - 1])
                if t < nmac:
                    phaseA(t, MACROS[t])

    nc.finalize()
    return nc


_NC_CACHE = {}


def _get_nc():
    if "nc" not in _NC_CACHE:
        _NC_CACHE["nc"] = _build_nc()
    return _NC_CACHE["nc"]


# --------------------------------------------------------------- host driver --
def _prep_weights(w1, g1, beta1, w2, b2, g2, beta2, w3, b3):
    C = np.eye(128, dtype=np.float64) - 1.0 / 128.0
    w1p = w1.astype(np.float64) @ C                       # [240,128]
    w2p = (g1.astype(np.float64)[:, None] * w2.astype(np.float64))
    b2c = beta1.astype(np.float64) @ w2.astype(np.float64) + b2.astype(np.float64)
    w3p = (g2.astype(np.float64)[:, None] * w3.astype(np.float64))
    b3c = beta2.astype(np.float64) @ w3.astype(np.float64) + b3.astype(np.float64)
    w1b_pad = np.zeros((128, 128), dtype=np.float64)
    w1b_pad[0:N_EQ_CH] = w1p[128:240]
    return {
        "w1a": np.ascontiguousarray(w1p[0:128]).astype(np.float16),
        "w1b": w1b_pad.astype(np.float16),
        "w2p": w2p.astype(np.float16),
        "w3p": w3p.astype(np.float16),
        "cmat": C.astype(np.float16),
        "onesd": np.full((128, 128), 1.0 / 128.0, dtype=np.float16),
        "ident": np.eye(128, dtype=np.float32),
        "identh": np.eye(128, dtype=np.float16),
        "ones1": np.ones((1, 128), dtype=np.float16),
        "b3row": np.tile(b3c, 8)[None, :].astype(np.float16),
        "b2c": b2c.astype(np.float32).reshape(128, 1),
    }


def _np_reference(ten, w1, g1, beta1, w2, b2, g2, beta2, w3, b3):
    """Pure-numpy fallback (used only if rep_layout is unexpected)."""
    x10 = ten[:, :N_INV]
    eq = ten[:, N_INV:]
    sumsq = np.zeros((ten.shape[0], N_EQ_CH), np.float32)
    for (nch, w, eqoff, choff, poff) in SEGS:
        sumsq[:, choff:choff + nch] = (
            (eq[:, eqoff:eqoff + nch * w].reshape(-1, nch, w) ** 2).sum(-1)
        )
    d = np.sqrt(sumsq + 1.0)
    x11 = d - 1.0
    x1 = np.concatenate([x10, x11], 1)
    seg = np.concatenate([np.repeat(np.arange(nch) + choff, w)
                          for (nch, w, eqoff, choff, poff) in SEGS])
    x2 = eq / d[:, seg]

    def ln(h, g, b):
        mu = h.mean(-1, keepdims=True)
        var = h.var(-1, keepdims=True)
        return (h - mu) / np.sqrt(var + EPS) * g + b

    h = x1 @ w1
    h = ln(h, g1, beta1)
    h = h @ w2 + b2
    h = h * (1.0 / (1.0 + np.exp(-h)))
    h = ln(h, g2, beta2)
    h = h @ w3 + b3
    return np.concatenate([h, x2], 1).astype(np.float32)


def kernel(ten, rep_layout, w1, g1, beta1, w2, b2, g2, beta2, w3, b3):
    ten = np.asarray(ten, dtype=np.float32)
    args = [np.asarray(a) for a in (w1, g1, beta1, w2, b2, g2, beta2, w3, b3)]
    w1, g1, beta1, w2, b2, g2, beta2, w3, b3 = [a.astype(np.float32) for a in args]

    if not np.array_equal(np.asarray(rep_layout).astype(np.int64), _EXPECTED_REP):
        return _np_reference(ten, w1, g1, beta1, w2, b2, g2, beta2, w3, b3)

    wmap = _prep_weights(w1, g1, beta1, w2, b2, g2, beta2, w3, b3)

    xpad = np.zeros((NPAD, DIM), dtype=np.float32)
    xpad[:N] = ten
    shards = xpad.reshape(N_CORES, ROWS_PER_CORE, DIM)

    nc = _get_nc()
    in_maps = [dict(wmap, x=np.ascontiguousarray(shards[c]))
               for c in range(N_CORES)]
    last_err = None
    for _attempt in range(3):
        try:
            res = run_bass_kernel_spmd(nc, in_maps, list(range(N_CORES))).results
            break
        except Exception as e:  # transient device-unrecoverable errors
            last_err = e
            import time as _time
            _time.sleep(10)
    else:
        raise last_err
    outp = np.concatenate([res[c]["out"] for c in range(N_CORES)], axis=0)
    return np.ascontiguousarray(outp[:N])


# revision 69
# speedup vs baseline: 1.0455x; 1.0078x over previous
"""Trainium2 Bass kernel for nn_EvMLP (segment_reduce EvNorm + invariant MLP).

Self-contained: hardcodes shapes/sharding. Accepts FULL inputs, returns FULL
output; shards the node dim N across 8 NeuronCores (pure data parallel).

Math (per row of ten [N, 592]):
  x10 = ten[:128]; eq = ten[128:]
  sumsq[c] = sum of eq^2 over segment c   (64x3, 32x5, 16x7 runs)
  d = sqrt(sumsq+1);  x11 = d-1;  x2 = eq / d[seg]
  x1 = [x10, x11]  (240)
  h = LN(x1@w1, g1, b1); h = silu(h@w2+b2); h = LN(h, g2, b2n); h = h@w3+b3
  out = [h, x2]

Implementation (fp16 compute, 5-phase software pipeline):
  - rows-on-partitions for the eq path; eq^2 written fp16 into a padded
    layout with a 1.0 slot per segment so the DVE segment reduce directly
    yields s1 = sumsq+1; work split across ACT/GP/DVE
  - rsqrt via bit-trick seed (i16 magic 0x59BA for fp16 / i32 0x5F3759DF for
    fp32 LN stats, computed on ACT) + ONE fused Newton step on DVE
  - MLP feature-major; all matmuls fp16 (4x faster PE than fp32); x10/x11
    transposed via PE into a shared PSUM ring, copy-cast to fp16 on ACT
  - LN mean folded into weights host-side; variance via ones/128 matmul;
    LN scales applied on DVE fused with the PSUM->SBUF move + fp16 cast
  - final bias b3 folded into the PE accumulation (rank-1 ones matmul)
  - per-macro work emitted as a 5-deep software pipeline (dma-in | eq+x2+
    transpose | H1+LN1 | H2+silu+AC+LN2 | H3+out-dma) with oldest work
    first, so every cross-phase dependency is >=1 iteration old and the
    in-order engine queues stream without same-iteration stalls; input
    DMAs issue from SP, output DMAs from GPSIMD so neither stream blocks
    the other
"""
import sys

sys.path.insert(0, "/opt/trn_rl_repo")

import numpy as np

import concourse.bass as bass
import concourse.bacc as bacc
import concourse.tile as tile
from concourse import mybir
from concourse.bass_utils import run_bass_kernel_spmd

f32 = mybir.dt.float32
f32r = mybir.dt.float32r
f16 = mybir.dt.float16
i32 = mybir.dt.int32
i16 = mybir.dt.int16

# ---------------------------------------------------------------- constants --
N = 100000
DIM = 592
N_INV = 128
N_EQ_CH = 112
N_EQ = 464
EPS = 1e-5
N_CORES = 8
BLOCKS_PER_CORE = 98                      # 98*128 = 12544 rows/core
ROWS_PER_CORE = BLOCKS_PER_CORE * 128
NPAD = N_CORES * ROWS_PER_CORE            # 100352
MACROS = [8] * 12 + [2]                   # blocks per macro-tile (sum 98)
MAGIC = 0x5F3759DF
MAGICF = float(MAGIC)
MAGIC16F = 22970.0                        # fp16 rsqrt seed magic (0x59BA)

# segment groups: (n_channels, width, eq col offset, channel offset, padded
# offset) — padded layout stores w+1 slots per channel, last slot == 1.0
SEGS = [(64, 3, 0, 0, 0), (32, 5, 192, 64, 256), (16, 7, 352, 96, 448)]
EQ2_PAD = 576                             # sum of nch*(w+1)

_EXPECTED_REP = np.concatenate(
    [np.repeat(np.arange(m) + off, 2 * l + 1)
     for l, (m, off) in enumerate([(128, 0), (64, 128), (32, 192), (16, 224)])]
)

# ------------------------------------------------------------- custom DVE op --
from concourse.dve_spec import (
    Spec, Src0, Src1, C0, C1, C2, lower, scan as dve_scan, sq as dve_sq, AluOp,
)
from concourse.dve_uop import DveOpSpec
import concourse.dve_ops as dve_ops
from concourse.dve_ops import DveOp

# Newton rsqrt step: out = y*(C1 - C0*((v+C2)*y*y));  in0=v, in1=y
_nr_body = Src1 * (C1 - ((Src0 + C2) * (Src1 * Src1)) * C0)


def _nr_ref(in0, in1, s0, s1, imm2):
    y = in1.astype(np.float32)
    v = in0.astype(np.float32)
    return (y * (np.float32(s1) - ((v + np.float32(imm2)) * y * y) * np.float32(s0))
            ).astype(np.float32)


def _register_op(name, spec):
    if name in dve_ops._SUB_OPCODE_FOR_NAME:
        for op in dve_ops.OPS:
            if op.name == name:
                return op
    from concourse.dve_spec import _has_src1 as has_src1
    shas = {}
    row = 1 + len(dve_ops.OPS)
    for ver in ("v3", "v4"):
        s = DveOpSpec(name=name, opcode=row, uops=lower(spec, ver=ver),
                      rd1_en=has_src1(spec))
        shas[ver] = s.sha(ver)
    op = DveOp(name, spec, subdim=False, uops_sha=shas)
    dve_ops.OPS.append(op)
    dve_ops._SUB_OPCODE_FOR_NAME[name] = row
    dve_ops.CUSTOM_DVE_SPECS[name] = spec
    return op


RSQRT_NR = _register_op("ANT_RSQRT_NR2", Spec(body=_nr_body, reference=_nr_ref))


def _mk_mulsub1():
    from concourse.dve_spec import One
    return _register_op(
        "ANT_MUL_SUB1",
        Spec(
            body=(Src0 * Src1) - One,
            reference=lambda in0, in1, s0, s1, imm2: (
                in0.astype(np.float32) * in1 - np.float32(1.0)
            ).astype(np.float32),
        ),
    )


MUL_SUB1 = _mk_mulsub1()

# running prefix sum of squares along the free stream (segment sums are
# recovered by differencing at segment boundaries)
SQ_PSUM = _register_op(
    "ANT_SQ_PSUM",
    Spec(
        body=dve_scan(AluOp.ADD, dve_sq(Src0)),
        reference=lambda in0, in1, s0, s1, imm2: np.cumsum(
            (in0.astype(np.float32) ** 2).reshape(in0.shape[0], -1),
            axis=-1, dtype=np.float32,
        ).reshape(in0.shape).astype(np.float32),
    ),
)


# ------------------------------------------------------------ kernel builder --
def _build_nc():
    nc = bacc.Bacc()

    x = nc.dram_tensor("x", [ROWS_PER_CORE, DIM], f32, kind="ExternalInput")
    out = nc.dram_tensor("out", [ROWS_PER_CORE, DIM], f32, kind="ExternalOutput")
    w1a_d = nc.dram_tensor("w1a", [128, 128], f16, kind="ExternalInput")
    w1b_d = nc.dram_tensor("w1b", [128, 128], f16, kind="ExternalInput")
    w2_d = nc.dram_tensor("w2p", [128, 128], f16, kind="ExternalInput")
    w3_d = nc.dram_tensor("w3p", [128, 128], f16, kind="ExternalInput")
    cmat_d = nc.dram_tensor("cmat", [128, 128], f16, kind="ExternalInput")
    onesd_d = nc.dram_tensor("onesd", [128, 128], f16, kind="ExternalInput")
    ident_d = nc.dram_tensor("ident", [128, 128], f32, kind="ExternalInput")
    identh_d = nc.dram_tensor("identh", [128, 128], f16, kind="ExternalInput")
    ones1_d = nc.dram_tensor("ones1", [1, 128], f16, kind="ExternalInput")
    b3row_d = nc.dram_tensor("b3row", [1, 1024], f16, kind="ExternalInput")
    b2_d = nc.dram_tensor("b2c", [128, 1], f32, kind="ExternalInput")

    # float consts used as activation bias
    for _v in (MAGICF, MAGIC16F):
        _t = nc.alloc_sbuf_tensor(f"const-f32-{_v}", [128, 1], f32)
        nc.gpsimd.memset(_t.ap(), _v)
        nc.const_aps.aps[(f32, _v)] = _t.ap()
    nc.all_engine_barrier()

    AF = mybir.ActivationFunctionType
    ALU = mybir.AluOpType
    AX = mybir.AxisListType

    from contextlib import ExitStack

    with tile.TileContext(nc) as tc:
        with ExitStack() as ctx:
            wpool = ctx.enter_context(tc.tile_pool(name="w", bufs=1))
            xpool = ctx.enter_context(tc.tile_pool(name="xp", bufs=3))
            opool = ctx.enter_context(tc.tile_pool(name="op", bufs=4))
            spool = ctx.enter_context(tc.tile_pool(name="sp", bufs=2))
            cpool = ctx.enter_context(tc.tile_pool(name="cp", bufs=2))
            ps_tp = ctx.enter_context(tc.tile_pool(name="ptp", bufs=1, space="PSUM"))
            ps_mm = ctx.enter_context(tc.tile_pool(name="pmm", bufs=2, space="PSUM"))
            ps_q = ctx.enter_context(tc.tile_pool(name="pq", bufs=1, space="PSUM"))

            def wload(name, shape, dtype, dram):
                t = wpool.tile(shape, dtype, tag=name)
                nc.gpsimd.dma_start(out=t, in_=dram[:, :])
                return t

            w1a = wload("w1a", [128, 128], f16, w1a_d)
            w1b = wload("w1b", [128, 128], f16, w1b_d)
            w2p = wload("w2p", [128, 128], f16, w2_d)
            w3p = wload("w3p", [128, 128], f16, w3_d)
            cmat = wload("cmat", [128, 128], f16, cmat_d)
            onesd = wload("onesd", [128, 128], f16, onesd_d)
            ident = wload("ident", [128, 128], f32, ident_d)
            identh = wload("identh", [128, 128], f16, identh_d)
            ones1 = wload("ones1", [1, 128], f16, ones1_d)
            b3row = wload("b3row", [1, 1024], f16, b3row_d)
            b2c = wload("b2c", [128, 1], f32, b2_d)

            # persistent ping-pong tiles: eq^2 padded (1.0 slot per segment
            # folds the +1 into the reduce) and x11 padded to 128 channels
            # (16 zero channels hit zero rows of w1b)
            eq2s = [wpool.tile([128, 8, EQ2_PAD], f16, tag=f"eq2{i}",
                               name=f"eq2{i}") for i in range(1)]
            x11s = [wpool.tile([128, 8, 128], f16, tag=f"x11{i}",
                               name=f"x11{i}") for i in range(2)]
            for t in eq2s:
                for (nch, w, eqoff, choff, poff) in SEGS:
                    pw = w + 1
                    ones_ap = t[:, :, poff : poff + nch * pw].rearrange(
                        "p b (c t) -> p b c t", t=pw
                    )[:, :, :, w : w + 1]
                    nc.gpsimd.memset(ones_ap, 1.0)
            for t in x11s:
                nc.gpsimd.memset(t[:, :, N_EQ_CH:128], 0.0)

            # software pipeline: emit eq-path(k) interleaved with MLP(k-1) so
            # each engine's in-order queue holds independent work from two
            # macros and cross-engine stalls overlap
            row_starts = []
            acc = 0
            for nb in MACROS:
                row_starts.append(acc)
                acc += nb * 128

            live = {}

            def phaseA(mi, nb):
                row0 = row_starts[mi]
                R_rows = nb * 128
                xv = x[row0 : row0 + R_rows, :].rearrange("(p b) d -> p b d", b=nb)
                X = xpool.tile([128, nb, DIM], f32, tag="X", name="X")
                nc.sync.dma_start(out=X, in_=xv)
                live[("X", mi)] = X

            def phaseB(mi, nb):
                row0 = row_starts[mi]
                R_rows = nb * 128
                RR = R_rows

                x11 = x11s[mi % 2]
                eq2 = eq2s[0]

                X = live.pop(("X", mi))
                O = opool.tile([128, nb, DIM], f32, tag="O", name="O")

                def sq_in(g):
                    nch, w, eqoff, choff, poff = SEGS[g]
                    return X[:, :, N_INV + eqoff : N_INV + eqoff + nch * w].rearrange(
                        "p b (c t) -> p b c t", t=w
                    )

                def sq_out(g):
                    nch, w, eqoff, choff, poff = SEGS[g]
                    return eq2[:, 0:nb, poff : poff + nch * (w + 1)].rearrange(
                        "p b (c t) -> p b c t", t=w + 1
                    )[:, :, :, 0:w]

                # ---- eq path (rows on partitions) ----
                nc.scalar.activation(out=sq_out(0), in_=sq_in(0), func=AF.Square)
                nc.gpsimd.tensor_tensor(
                    out=sq_out(1), in0=sq_in(1), in1=sq_in(1), op=ALU.mult
                )
                nc.scalar.activation(out=sq_out(2), in_=sq_in(2), func=AF.Square)

                # segment reduce (fp16 in/out, 1.0 pad slot folds in the +1)
                s1 = spool.tile([128, nb, N_EQ_CH], f16, tag="s1")
                with nc.allow_low_precision("fp16 segment sumsq; tol 2e-2"):
                    for (nch, w, eqoff, choff, poff) in SEGS:
                        pw = w + 1
                        nc.vector.reduce_sum(
                            out=s1[:, :, choff : choff + nch],
                            in_=eq2[:, 0:nb, poff : poff + nch * pw].rearrange(
                                "p b (c t) -> p b c t", t=pw
                            ),
                            axis=AX.X,
                        )

                # r = rsqrt(s1): i16 bit-trick seed on ACT + 1 Newton on DVE
                seedb = spool.tile([128, nb, N_EQ_CH], i16, tag="seedb")
                nc.scalar.activation(
                    out=seedb, in_=s1.bitcast(i16), func=AF.Identity,
                    scale=-0.5, bias=MAGIC16F,
                )
                flat3 = lambda ap: ap.rearrange("p a b -> p (a b)")
                r = spool.tile([128, nb, N_EQ_CH], f16, tag="r")
                nc.vector._custom_dve(
                    RSQRT_NR, out=flat3(r), in0=flat3(s1),
                    in1=flat3(seedb.bitcast(f16)), s0=0.5, s1=1.5, imm2=0.0,
                )
                # x11 = s1*r - 1  (= sqrt(s1) - 1), fp16, into padded tile
                nc.vector._custom_dve(
                    MUL_SUB1, out=x11[:, 0:nb, 0:N_EQ_CH], in0=s1,
                    in1=r, s0=0.0, s1=0.0, imm2=0.0,
                )

                # x2 = eq * r[seg] -> O[:, :, 128:]  (fp32 out for DMA)
                for g, eng in ((0, nc.vector), (1, nc.gpsimd), (2, nc.gpsimd)):
                    nch, w, eqoff, choff, poff = SEGS[g]
                    rbc = (
                        r[:, :, choff : choff + nch]
                        .unsqueeze(-1)
                        .broadcast_to((128, nb, nch, w))
                    )
                    eng.tensor_tensor(
                        out=O[:, :, N_INV + eqoff : N_INV + eqoff + nch * w].rearrange(
                            "p b (c t) -> p b c t", t=w
                        ),
                        in0=sq_in(g),
                        in1=rbc,
                        op=ALU.mult,
                    )
                live[("O", mi)] = O

                # x10^T via PE into a macro PSUM tile, one ACT copy-cast; then
                # x11^T (fp16) into the same PSUM ring slot, one ACT copy
                TPa = ps_tp.tile([128, RR], f32, tag="tp", name="TPa")
                for b in range(nb):
                    nc.tensor.transpose(
                        TPa[:, b * 128 : (b + 1) * 128], X[:, b, 0:N_INV], ident
                    )
                x1Ta = cpool.tile([128, RR], f16, tag="x1Ta", name="x1Ta")
                nc.scalar.activation(out=x1Ta, in_=TPa, func=AF.Identity)

                TPb = ps_tp.tile([128, RR], f16, tag="tp", name="TPb")
                for b in range(nb):
                    nc.tensor.transpose(
                        TPb[:, b * 128 : (b + 1) * 128], x11[:, b, :], identh
                    )
                x1Tb = cpool.tile([128, RR], f16, tag="x1Tb", name="x1Tb")
                nc.scalar.activation(out=x1Tb, in_=TPb, func=AF.Identity)

                live[mi] = (nb, x1Ta, x1Tb)

            def phaseC(mi, nb):
                RR = nb * 128
                _, x1Ta, x1Tb = live[mi]
                nbanks = (RR + 511) // 512

                # H1 = w1a^T x10^T + w1b^T x11^T   [128, RR] PSUM f32
                # (grouped by stationary weight so PE can reuse loads)
                H1 = ps_mm.tile([128, RR], f32, tag="mm", name="H1")
                for c in range(nbanks):
                    lo = c * 512
                    hi = min(RR, lo + 512)
                    nc.tensor.matmul(
                        H1[:, lo:hi], w1a, x1Ta[:, lo:hi], start=True, stop=False
                    )
                for c in range(nbanks):
                    lo = c * 512
                    hi = min(RR, lo + 512)
                    nc.tensor.matmul(
                        H1[:, lo:hi], w1b, x1Tb[:, lo:hi],
                        start=False, stop=True,
                    )

                # LN1 stats: sq1 -> Q1 = mean(sq1); rstd1 = rsqrt(Q1+eps)
                sq1 = cpool.tile([128, RR], f16, tag="sq1")
                nc.scalar.activation(out=sq1, in_=H1, func=AF.Square)
                Q1 = ps_q.tile([128, RR], f32, tag="q")
                for c in range(nbanks):
                    lo = c * 512
                    hi = min(RR, lo + 512)
                    nc.tensor.matmul(
                        Q1[:, lo:hi], onesd, sq1[:, lo:hi], start=True, stop=True
                    )
                sd1 = cpool.tile([128, RR], i32, tag="sd1", bufs=1)
                nc.scalar.activation(out=sd1, in_=Q1.bitcast(i32),
                                     func=AF.Identity, scale=-0.5, bias=MAGICF)
                rstd1 = cpool.tile([128, RR], f16, tag="rstd1")
                nc.vector._custom_dve(
                    RSQRT_NR, out=rstd1, in0=Q1, in1=sd1.bitcast(f32),
                    s0=0.5, s1=1.5, imm2=float(EPS),
                )
                # hm1 = H1 * rstd1 (fused PSUM->SBUF move + fp16 cast)
                hm1 = cpool.tile([128, RR], f16, tag="hm1")
                nc.vector.tensor_tensor(out=hm1, in0=H1, in1=rstd1, op=ALU.mult)
                live[mi] = (nb, hm1)

            def phaseD(mi, nb):
                RR = nb * 128
                _, hm1 = live.pop(mi)
                nbanks = (RR + 511) // 512

                # H2 = w2p^T hm1 ; avs = silu(H2 + b2c)
                H2 = ps_mm.tile([128, RR], f32, tag="mm")
                for c in range(nbanks):
                    lo = c * 512
                    hi = min(RR, lo + 512)
                    nc.tensor.matmul(
                        H2[:, lo:hi], w2p, hm1[:, lo:hi], start=True, stop=True
                    )
                avs = cpool.tile([128, RR], f16, tag="avs")
                nc.scalar.activation(out=avs, in_=H2, func=AF.Silu, bias=b2c)

                # AC = cmat^T avs (mean-centered); LN2 stats
                AC = ps_mm.tile([128, RR], f32, tag="mm")
                for c in range(nbanks):
                    lo = c * 512
                    hi = min(RR, lo + 512)
                    nc.tensor.matmul(
                        AC[:, lo:hi], cmat, avs[:, lo:hi], start=True, stop=True
                    )
                sq2 = cpool.tile([128, RR], f16, tag="sq2")
                nc.scalar.activation(out=sq2, in_=AC, func=AF.Square)
                Q2 = ps_q.tile([128, RR], f32, tag="q")
                for c in range(nbanks):
                    lo = c * 512
                    hi = min(RR, lo + 512)
                    nc.tensor.matmul(
                        Q2[:, lo:hi], onesd, sq2[:, lo:hi], start=True, stop=True
                    )
                sd2 = cpool.tile([128, RR], i32, tag="sd2", bufs=1)
                nc.scalar.activation(out=sd2, in_=Q2.bitcast(i32),
                                     func=AF.Identity, scale=-0.5, bias=MAGICF)
                rstd2 = cpool.tile([128, RR], f16, tag="rstd2")
                nc.vector._custom_dve(
                    RSQRT_NR, out=rstd2, in0=Q2, in1=sd2.bitcast(f32),
                    s0=0.5, s1=1.5, imm2=float(EPS),
                )
                hn2 = cpool.tile([128, RR], f16, tag="hn2")
                nc.vector.tensor_tensor(out=hn2, in0=AC, in1=rstd2, op=ALU.mult)
                live[("hn2", mi)] = hn2

            def phaseE(mi, nb):
                row0 = row_starts[mi]
                R_rows = nb * 128
                RR = R_rows
                hn2 = live.pop(("hn2", mi))
                nbanks = (RR + 511) // 512

                # H3 natural orientation: bias via rank-1 ones matmul, then
                # per-block lhsT=hn2 matmuls accumulate on top
                H3n = ps_q.tile([128, RR], f32, tag="q")
                for c in range(nbanks):
                    lo = c * 512
                    hi = min(RR, lo + 512)
                    nc.tensor.matmul(
                        H3n[:, lo:hi], ones1, b3row[:, lo:hi],
                        start=True, stop=False,
                    )
                    for j in range(lo // 128, hi // 128):
                        nc.tensor.matmul(
                            H3n[:, j * 128 : (j + 1) * 128],
                            hn2[:, j * 128 : (j + 1) * 128], w3p,
                            start=False, stop=True,
                        )
                O = live.pop(("O", mi))
                nc.scalar.activation(
                    out=O[:, :, 0:N_INV],
                    in_=H3n.rearrange("p (b j) -> p b j", j=128),
                    func=AF.Identity,
                )
                ov = out[row0 : row0 + R_rows, :].rearrange("(p b) d -> p b d", b=nb)
                nc.gpsimd.dma_start(out=ov, in_=O)

            # 5-deep software pipeline, oldest work emitted first; every
            # cross-phase dependency is >= 1 iteration old so each engine's
            # in-order queue streams without same-iteration stalls
            nmac = len(MACROS)
            for t in range(nmac + 4):
                if 4 <= t:
                    phaseE(t - 4, MACROS[t - 4])
                if 3 <= t <= nmac + 2:
                    phaseD(t - 3, MACROS[t - 3])
                if 2 <= t <= nmac + 1:
                    phaseC(t - 2, MACROS[t - 2])
                if 1 <= t <= nmac:
                    phaseB(t - 1, MACROS[t - 1])
                if t < nmac:
                    phaseA(t, MACROS[t])

    nc.finalize()
    return nc


_NC_CACHE = {}


def _get_nc():
    if "nc" not in _NC_CACHE:
        _NC_CACHE["nc"] = _build_nc()
    return _NC_CACHE["nc"]


# --------------------------------------------------------------- host driver --
def _prep_weights(w1, g1, beta1, w2, b2, g2, beta2, w3, b3):
    C = np.eye(128, dtype=np.float64) - 1.0 / 128.0
    w1p = w1.astype(np.float64) @ C                       # [240,128]
    w2p = (g1.astype(np.float64)[:, None] * w2.astype(np.float64))
    b2c = beta1.astype(np.float64) @ w2.astype(np.float64) + b2.astype(np.float64)
    w3p = (g2.astype(np.float64)[:, None] * w3.astype(np.float64))
    b3c = beta2.astype(np.float64) @ w3.astype(np.float64) + b3.astype(np.float64)
    w1b_pad = np.zeros((128, 128), dtype=np.float64)
    w1b_pad[0:N_EQ_CH] = w1p[128:240]
    return {
        "w1a": np.ascontiguousarray(w1p[0:128]).astype(np.float16),
        "w1b": w1b_pad.astype(np.float16),
        "w2p": w2p.astype(np.float16),
        "w3p": w3p.astype(np.float16),
        "cmat": C.astype(np.float16),
        "onesd": np.full((128, 128), 1.0 / 128.0, dtype=np.float16),
        "ident": np.eye(128, dtype=np.float32),
        "identh": np.eye(128, dtype=np.float16),
        "ones1": np.ones((1, 128), dtype=np.float16),
        "b3row": np.tile(b3c, 8)[None, :].astype(np.float16),
        "b2c": b2c.astype(np.float32).reshape(128, 1),
    }


def _np_reference(ten, w1, g1, beta1, w2, b2, g2, beta2, w3, b3):
    """Pure-numpy fallback (used only if rep_layout is unexpected)."""
    x10 = ten[:, :N_INV]
    eq = ten[:, N_INV:]
    sumsq = np.zeros((ten.shape[0], N_EQ_CH), np.float32)
    for (nch, w, eqoff, choff, poff) in SEGS:
        sumsq[:, choff:choff + nch] = (
            (eq[:, eqoff:eqoff + nch * w].reshape(-1, nch, w) ** 2).sum(-1)
        )
    d = np.sqrt(sumsq + 1.0)
    x11 = d - 1.0
    x1 = np.concatenate([x10, x11], 1)
    seg = np.concatenate([np.repeat(np.arange(nch) + choff, w)
                          for (nch, w, eqoff, choff, poff) in SEGS])
    x2 = eq / d[:, seg]

    def ln(h, g, b):
        mu = h.mean(-1, keepdims=True)
        var = h.var(-1, keepdims=True)
        return (h - mu) / np.sqrt(var + EPS) * g + b

    h = x1 @ w1
    h = ln(h, g1, beta1)
    h = h @ w2 + b2
    h = h * (1.0 / (1.0 + np.exp(-h)))
    h = ln(h, g2, beta2)
    h = h @ w3 + b3
    return np.concatenate([h, x2], 1).astype(np.float32)


def kernel(ten, rep_layout, w1, g1, beta1, w2, b2, g2, beta2, w3, b3):
    ten = np.asarray(ten, dtype=np.float32)
    args = [np.asarray(a) for a in (w1, g1, beta1, w2, b2, g2, beta2, w3, b3)]
    w1, g1, beta1, w2, b2, g2, beta2, w3, b3 = [a.astype(np.float32) for a in args]

    if not np.array_equal(np.asarray(rep_layout).astype(np.int64), _EXPECTED_REP):
        return _np_reference(ten, w1, g1, beta1, w2, b2, g2, beta2, w3, b3)

    wmap = _prep_weights(w1, g1, beta1, w2, b2, g2, beta2, w3, b3)

    xpad = np.zeros((NPAD, DIM), dtype=np.float32)
    xpad[:N] = ten
    shards = xpad.reshape(N_CORES, ROWS_PER_CORE, DIM)

    nc = _get_nc()
    in_maps = [dict(wmap, x=np.ascontiguousarray(shards[c]))
               for c in range(N_CORES)]
    last_err = None
    for _attempt in range(3):
        try:
            res = run_bass_kernel_spmd(nc, in_maps, list(range(N_CORES))).results
            break
        except Exception as e:  # transient device-unrecoverable errors
            last_err = e
            import time as _time
            _time.sleep(10)
    else:
        raise last_err
    outp = np.concatenate([res[c]["out"] for c in range(N_CORES)], axis=0)
    return np.ascontiguousarray(outp[:N])
